# revision 4
# baseline (speedup 1.0000x reference)
"""Trainium2 Bass kernel for 0.7*BCEWithLogits + 0.3*MultiLabelMarginLoss.

Math (per row of N = B*T rows, V = 128 classes; output = mean over rows):
  bce_row = (1/V) [ sum_n softplus(x_n) - sum_n x_n t_n ]
  mlm_row = (1/V) sum_{p in pos} sum_{n not in pos} relu(1 - x_p + x_n)

Host prep (numpy, like the npos-sort the previous version already did):
  u = x with positive positions replaced by -15        [N, V]
  v = per-row table of negated positive logits (-x_p), padded to W
      slots with -15                                   [N, W]

Device math per row then collapses to two primitives:
  * softplus sum: one Exp pass + one Ln(1+e) pass with accum over the
    whole (u|v) slab. u-positives are -15 -> contribute ~0; each real
    table slot contributes softplus(-x_p) = softplus(x_p) - x_p, which
    is exactly the missing positive softplus term AND the -x*t BCE term
    in one shot; pads contribute softplus(-15) ~ 3e-7.
  * hinge: z[p,k,n] = relu(u_n + v_k + 1) with accum -> per-row
    sum_k sum_n relu(1 - x_p + x_n) over negatives only. No select
    needed: pads (v=-15) and positive n-positions (u=-15) both push the
    argument below -8 so relu kills them.

Loss = [0.7*(ln-accums) + 0.3*(hinge-accums)] / (V*N) summed over rows;
per-core partial reduced on device to a single scalar via ones-matmul.

Sharding: host sorts rows by npos DESCENDING, deals them round-robin to
the 8 cores (identical npos profile per core), packs each core's 16
blocks side-by-side as [128, NBLK*(V+W)] bf16 so each 4-block group is
one contiguous DMA. Block b's hinge uses S_b = max npos in the block
(data-derived schedule, one cached NEFF per distinct schedule).
bf16 end-to-end (halves DMA); accumulations are f32 on-engine.
"""

import sys

sys.path.insert(0, "/opt/trn_rl_repo")

import ml_dtypes
import numpy as np

import concourse.bacc as bacc
import concourse.tile as tile
from concourse import mybir
from concourse.bass_utils import run_bass_kernel_spmd

F32 = mybir.dt.float32
BF16 = mybir.dt.bfloat16
ALU = mybir.AluOpType
ACTF = mybir.ActivationFunctionType
AXL = mybir.AxisListType

B, T, V = 16, 1024, 128
ROWS = B * T
N_CORES = 8
RPC = ROWS // N_CORES             # 2048 rows per core
P = 128                           # rows per block
NBLK = RPC // P                   # 16 blocks
GRP = 4                           # blocks per group (one DMA per group)
NGRP = NBLK // GRP

NEG = -15.0                       # positive-position / pad fill value
BCE_W = 0.7
MLM_W = 0.3

CHUNKS = (1, 5, 5, 5)             # DMA chunk sizes in blocks (small first)
OFFLOAD = 5                       # trailing small-S blocks hinged on ACT


def _register_ops():
    from concourse import dve_ops as dops
    from concourse.dve_spec import Spec, Src0, Src1, AluOp, relu, C1

    if hasattr(dops, "ANT_HINGE_OP"):
        return dops.ANT_HINGE_OP

    def _zref(in0, in1, c0, c1, c2):
        i0 = in0.astype(np.float32).reshape(in0.shape[0], -1)
        t = in1.astype(np.float32).reshape(in1.shape[0], -1)
        b = np.maximum(i0 + t + c1, 0.0)
        return b, b.sum(-1, keepdims=True)

    z_spec = Spec(
        body=relu(Src0 + Src1 + C1),
        accum=AluOp.ADD, reference=_zref,
    )

    name = "Z_HINGE_ADD_ANT"
    opc = max(dops._SUB_OPCODE_FOR_NAME.values()) + 1
    shas = {}
    for ver in ("v3", "v4"):
        r = dops.DveOpSpec(
            name=name, opcode=opc,
            uops=dops.lower(z_spec, ver=ver), rd1_en=dops.has_src1(z_spec),
        )
        shas[ver] = r.sha(ver)
    op = dops.DveOp(name, z_spec, subdim=False, uops_sha=shas)
    dops.OPS.append(op)
    dops.CUSTOM_DVE_SPECS[name] = z_spec
    dops._SUB_OPCODE_FOR_NAME[name] = opc
    dops.ANT_HINGE_OP = op
    return op


Z_HINGE = _register_ops()


def _act_set_id(nc):
    from concourse.hw_specs import get_activation_tables

    return list(get_activation_tables(nc.m.arch)).index("natural_log_exp_and_others")


def build_nc(schedule, W):
    """schedule: per-block hinge-slot counts (desc); W: table width (cols)."""
    CB = V + W
    n_dve = NBLK - OFFLOAD
    n_off = sum(schedule[n_dve:])          # slot count offloaded to ACT
    nc = bacc.Bacc("TRN2", target_bir_lowering=False, debug=False)
    xp_dram = nc.dram_tensor("xp", [P, NBLK * CB], BF16, kind="ExternalInput")
    out_dram = nc.dram_tensor("out", [1, 1], F32, kind="ExternalOutput")
    xp_ap = xp_dram.ap()

    # chunk -> (first block, n blocks); block -> (chunk, tile col offset)
    chunk_of = {}
    b0 = 0
    for ci, nb in enumerate(CHUNKS):
        for j in range(nb):
            chunk_of[b0 + j] = (ci, j * CB)
        b0 += nb

    with tile.TileContext(nc) as tc:
        with (
            tc.tile_pool(name="const", bufs=1) as cpool,
            tc.tile_pool(name="inp", bufs=1) as ipool,       # full residency
            tc.tile_pool(name="act", bufs=2) as apool_e,
            tc.tile_pool(name="so", bufs=2) as spool,
            tc.tile_pool(name="zp", bufs=2) as zpool,
            tc.tile_pool(name="accs", bufs=1) as apool,
            tc.tile_pool(name="ps", bufs=1, space="PSUM") as pspool,
        ):
            nc.scalar.add_instruction(
                mybir.InstLoadActFuncSet(
                    name=nc.get_next_instruction_name(), ins=[], outs=[],
                    act_func_set_id=_act_set_id(nc),
                )
            )
            ones = cpool.tile([P, 1], F32, tag="ones")
            nc.vector.memset(ones[:], 1.0)
            # hinge accums: DVE blocks then ACT-offloaded slots, one reduce
            allc = apool.tile([P, n_dve + n_off], F32, tag="allc")
            lcols = apool.tile([P, len(CHUNKS)], F32, tag="lcols")
            v1 = apool.tile([P, max(1, n_off)], F32, tag="v1")

            ctiles = []
            b0 = 0
            for ci, nb in enumerate(CHUNKS):
                cw = nb * CB
                xg = ipool.tile([P, cw], BF16, tag=f"c{ci}")
                nc.sync.dma_start(
                    xg[:], xp_ap[:, b0 * CB : b0 * CB + cw]
                )
                ctiles.append(xg)
                # softplus: Exp then Ln(1+e) with accum, chunk incl tables
                eg = apool_e.tile([P, cw], BF16, tag=f"e{ci}")
                nc.scalar.activation(eg[:], xg[:], ACTF.Exp, bias=0.0, scale=1.0)
                lg = apool_e.tile([P, cw], BF16, tag=f"l{ci}")
                nc.scalar.activation(
                    lg[:], eg[:], ACTF.Ln, bias=1.0, scale=1.0,
                    accum_out=lcols[:, ci : ci + 1],
                )
                b0 += nb

            # fused hinge on DVE for the big blocks
            for blk in range(n_dve):
                S = schedule[blk]
                ci, c0 = chunk_of[blk]
                xg = ctiles[ci]
                u = xg[:, c0 : c0 + V]
                v = xg[:, c0 + V : c0 + V + S]
                zr = zpool.tile([P, S * V], BF16, tag="zr")
                zv = zr[:].rearrange("p (s n) -> p s n", s=S)
                u_b = u.unsqueeze(1).broadcast_to([P, S, V])
                v_b = v.unsqueeze(2).broadcast_to([P, S, V])
                nc.vector._custom_dve(
                    Z_HINGE, out=zv, in0=u_b, in1=v_b,
                    s0=0.0, s1=1.0,
                    accum_out=allc[:, blk : blk + 1],
                )

            # hinge on ACT for the small tail blocks: per slot,
            # relu(u + (1 + v_k)) with per-partition bias
            si = 0
            for blk in range(n_dve, NBLK):
                S = schedule[blk]
                ci, c0 = chunk_of[blk]
                xg = ctiles[ci]
                nc.scalar.activation(
                    v1[:, si : si + S], xg[:, c0 + V : c0 + V + S],
                    ACTF.Copy, bias=1.0, scale=1.0,
                )
                for k in range(S):
                    so = spool.tile([P, V], BF16, tag="so")
                    nc.scalar.activation(
                        so[:], xg[:, c0 : c0 + V], ACTF.Relu,
                        bias=v1[:, si : si + 1], scale=1.0,
                        accum_out=allc[:, n_dve + si : n_dve + si + 1],
                    )
                    si += 1

            # ---- end-of-core combine ----
            h1 = apool.tile([P, 1], F32, tag="h1")
            nc.vector.tensor_reduce(h1[:], allc[:], AXL.X, ALU.add)
            l1 = apool.tile([P, 1], F32, tag="l1")
            nc.vector.tensor_reduce(l1[:], lcols[:], AXL.X, ALU.add)
            # w = (0.3/0.7)*h1 + l1 per partition; host applies 0.7/(V*ROWS)
            w1 = apool.tile([P, 1], F32, tag="w1")
            nc.vector.scalar_tensor_tensor(
                w1[:], h1[:], MLM_W / BCE_W, l1[:], ALU.mult, ALU.add
            )
            wps = pspool.tile([1, 1], F32, tag="wps")
            nc.tensor.matmul(wps[:], ones[:], w1[:], start=True, stop=True)
            wsb = apool.tile([1, 1], F32, tag="wsb")
            nc.scalar.copy(wsb[:], wps[:])
            nc.sync.dma_start(out_dram.ap()[:, :], wsb[:])

    nc.compile()
    return nc


_NC_CACHE = {}


def _get_nc(schedule, W):
    key = (schedule, W)
    if key not in _NC_CACHE:
        _NC_CACHE[key] = build_nc(schedule, W)
    return _NC_CACHE[key]


def _shard(x, t):
    """npos-sorted (descending) round-robin shard.
    Returns (schedule, W, [per-core [P, NBLK*(V+W)] bf16 arrays])."""
    pos = t > 0.5
    npos = pos.sum(axis=1)
    order = np.argsort(-npos, kind="stable")
    npos_s = npos[order]
    schedule = tuple(max(1, int(npos_s[b * N_CORES * P])) for b in range(NBLK))
    W = max(2, (schedule[0] + 1) // 2 * 2)     # even table width >= max S
    CB = V + W

    xs = x[order]
    ps = pos[order]
    u = np.where(ps, np.float32(NEG), xs)
    pcols = np.argsort(~ps, axis=1, kind="stable")[:, :W]
    vals = -np.take_along_axis(xs, pcols, axis=1)
    valid = np.arange(W)[None, :] < npos_s[:, None]
    v = np.where(valid, vals, np.float32(NEG))
    slab = np.concatenate([u, v], axis=1).astype(ml_dtypes.bfloat16)  # [ROWS, CB]

    shards = []
    for c in range(N_CORES):
        s = slab[c::N_CORES]                      # [RPC, CB] desc-sorted
        s = s.reshape(NBLK, P, CB).transpose(1, 0, 2).reshape(P, NBLK * CB)
        shards.append(np.ascontiguousarray(s))
    return schedule, W, shards


def kernel(logits: np.ndarray, targets: np.ndarray) -> np.ndarray:
    x = np.asarray(logits, dtype=np.float32).reshape(ROWS, V)
    t = np.asarray(targets, dtype=np.float32).reshape(ROWS, V)
    schedule, W, shards = _shard(x, t)
    nc = _get_nc(schedule, W)
    in_maps = [{"xp": shards[c]} for c in range(N_CORES)]
    res = run_bass_kernel_spmd(nc, in_maps, list(range(N_CORES)))
    total = sum(float(res.results[c]["out"][0, 0]) for c in range(N_CORES))
    return np.float32(total * BCE_W / (V * ROWS))


# revision 5
# speedup vs baseline: 1.1079x; 1.1079x over previous
"""Trainium2 Bass kernel for 0.7*BCEWithLogits + 0.3*MultiLabelMarginLoss.

Math (per row of N = B*T rows, V = 128 classes; output = mean over rows):
  bce_row = (1/V) [ sum_n softplus(x_n) - sum_n x_n t_n ]
  mlm_row = (1/V) sum_{p in pos} sum_{n not in pos} relu(1 - x_p + x_n)

Host prep (numpy, like the npos-sort the previous version already did):
  u = x with positive positions replaced by -15        [N, V]
  v = per-row table of negated positive logits (-x_p), padded to W
      slots with -15                                   [N, W]

Device math per row then collapses to two primitives:
  * softplus sum: one Exp pass + one Ln(1+e) pass with accum over the
    whole (u|v) slab. u-positives are -15 -> contribute ~0; each real
    table slot contributes softplus(-x_p) = softplus(x_p) - x_p, which
    is exactly the missing positive softplus term AND the -x*t BCE term
    in one shot; pads contribute softplus(-15) ~ 3e-7.
  * hinge: z[p,k,n] = relu(u_n + v_k + 1) with accum -> per-row
    sum_k sum_n relu(1 - x_p + x_n) over negatives only. No select
    needed: pads (v=-15) and positive n-positions (u=-15) both push the
    argument below -8 so relu kills them.

Loss = [0.7*(ln-accums) + 0.3*(hinge-accums)] / (V*N) summed over rows;
per-core partial reduced on device to a single scalar via ones-matmul.

Sharding: host sorts rows by npos DESCENDING, deals them round-robin to
the 8 cores (identical npos profile per core), packs each core's 16
blocks side-by-side as [128, NBLK*(V+W)] bf16 so each 4-block group is
one contiguous DMA. Block b's hinge uses S_b = max npos in the block
(data-derived schedule, one cached NEFF per distinct schedule).
bf16 end-to-end (halves DMA); accumulations are f32 on-engine.
"""

import sys

sys.path.insert(0, "/opt/trn_rl_repo")

import ml_dtypes
import numpy as np

import concourse.bacc as bacc
import concourse.tile as tile
from concourse import mybir
from concourse.bass_utils import run_bass_kernel_spmd

F32 = mybir.dt.float32
BF16 = mybir.dt.bfloat16
ALU = mybir.AluOpType
ACTF = mybir.ActivationFunctionType
AXL = mybir.AxisListType

B, T, V = 16, 1024, 128
ROWS = B * T
N_CORES = 8
RPC = ROWS // N_CORES             # 2048 rows per core
P = 128                           # rows per block
NBLK = RPC // P                   # 16 blocks
GRP = 4                           # blocks per group (one DMA per group)
NGRP = NBLK // GRP

NEG = -15.0                       # positive-position / pad fill value
BCE_W = 0.7
MLM_W = 0.3

CHUNKS = (1, 5, 5, 5)             # DMA chunk sizes in blocks (small first)
OFFLOAD = 0                       # trailing small-S blocks hinged on ACT
                                  # (measured: ACT slot = ACTIVATE ~355ns +
                                  # READ_ACCUMULATOR ~278ns, 5.7x DVE's cost
                                  # per slot -> offload loses)


def _register_ops():
    from concourse import dve_ops as dops
    from concourse.dve_spec import Spec, Src0, Src1, AluOp, relu, C1

    if hasattr(dops, "ANT_HINGE_OP"):
        return dops.ANT_HINGE_OP

    def _zref(in0, in1, c0, c1, c2):
        i0 = in0.astype(np.float32).reshape(in0.shape[0], -1)
        t = in1.astype(np.float32).reshape(in1.shape[0], -1)
        b = np.maximum(i0 + t + c1, 0.0)
        return b, b.sum(-1, keepdims=True)

    z_spec = Spec(
        body=relu(Src0 + Src1 + C1),
        accum=AluOp.ADD, reference=_zref,
    )

    name = "Z_HINGE_ADD_ANT"
    opc = max(dops._SUB_OPCODE_FOR_NAME.values()) + 1
    shas = {}
    for ver in ("v3", "v4"):
        r = dops.DveOpSpec(
            name=name, opcode=opc,
            uops=dops.lower(z_spec, ver=ver), rd1_en=dops.has_src1(z_spec),
        )
        shas[ver] = r.sha(ver)
    op = dops.DveOp(name, z_spec, subdim=False, uops_sha=shas)
    dops.OPS.append(op)
    dops.CUSTOM_DVE_SPECS[name] = z_spec
    dops._SUB_OPCODE_FOR_NAME[name] = opc
    dops.ANT_HINGE_OP = op
    return op


Z_HINGE = _register_ops()


def _act_set_id(nc):
    from concourse.hw_specs import get_activation_tables

    return list(get_activation_tables(nc.m.arch)).index("natural_log_exp_and_others")


def build_nc(schedule, W):
    """schedule: per-block hinge-slot counts (desc); W: table width (cols)."""
    CB = V + W
    n_dve = NBLK - OFFLOAD
    n_off = sum(schedule[n_dve:])          # slot count offloaded to ACT
    nc = bacc.Bacc("TRN2", target_bir_lowering=False, debug=False)
    xp_dram = nc.dram_tensor("xp", [P, NBLK * CB], BF16, kind="ExternalInput")
    out_dram = nc.dram_tensor("out", [1, 1], F32, kind="ExternalOutput")
    xp_ap = xp_dram.ap()

    # chunk -> (first block, n blocks); block -> (chunk, tile col offset)
    chunk_of = {}
    b0 = 0
    for ci, nb in enumerate(CHUNKS):
        for j in range(nb):
            chunk_of[b0 + j] = (ci, j * CB)
        b0 += nb

    with tile.TileContext(nc) as tc:
        with (
            tc.tile_pool(name="const", bufs=1) as cpool,
            tc.tile_pool(name="inp", bufs=1) as ipool,       # full residency
            tc.tile_pool(name="act", bufs=2) as apool_e,
            tc.tile_pool(name="so", bufs=2) as spool,
            tc.tile_pool(name="zp", bufs=2) as zpool,
            tc.tile_pool(name="accs", bufs=1) as apool,
            tc.tile_pool(name="ps", bufs=1, space="PSUM") as pspool,
        ):
            nc.scalar.add_instruction(
                mybir.InstLoadActFuncSet(
                    name=nc.get_next_instruction_name(), ins=[], outs=[],
                    act_func_set_id=_act_set_id(nc),
                )
            )
            ones = cpool.tile([P, 1], F32, tag="ones")
            nc.vector.memset(ones[:], 1.0)
            # hinge accums: DVE blocks then ACT-offloaded slots, one reduce
            allc = apool.tile([P, n_dve + n_off], F32, tag="allc")
            lcols = apool.tile([P, len(CHUNKS)], F32, tag="lcols")
            v1 = apool.tile([P, max(1, n_off)], F32, tag="v1")

            ctiles = []
            b0 = 0
            for ci, nb in enumerate(CHUNKS):
                cw = nb * CB
                xg = ipool.tile([P, cw], BF16, tag=f"c{ci}")
                nc.sync.dma_start(
                    xg[:], xp_ap[:, b0 * CB : b0 * CB + cw]
                )
                ctiles.append(xg)
                # softplus: Exp then Ln(1+e) with accum, chunk incl tables
                eg = apool_e.tile([P, cw], BF16, tag=f"e{ci}")
                nc.scalar.activation(eg[:], xg[:], ACTF.Exp, bias=0.0, scale=1.0)
                lg = apool_e.tile([P, cw], BF16, tag=f"l{ci}")
                nc.scalar.activation(
                    lg[:], eg[:], ACTF.Ln, bias=1.0, scale=1.0,
                    accum_out=lcols[:, ci : ci + 1],
                )
                b0 += nb

            # fused hinge on DVE for the big blocks
            for blk in range(n_dve):
                S = schedule[blk]
                ci, c0 = chunk_of[blk]
                xg = ctiles[ci]
                u = xg[:, c0 : c0 + V]
                v = xg[:, c0 + V : c0 + V + S]
                zr = zpool.tile([P, S * V], BF16, tag="zr")
                zv = zr[:].rearrange("p (s n) -> p s n", s=S)
                u_b = u.unsqueeze(1).broadcast_to([P, S, V])
                v_b = v.unsqueeze(2).broadcast_to([P, S, V])
                nc.vector._custom_dve(
                    Z_HINGE, out=zv, in0=u_b, in1=v_b,
                    s0=0.0, s1=1.0,
                    accum_out=allc[:, blk : blk + 1],
                )

            # hinge on ACT for the small tail blocks: per slot,
            # relu(u + (1 + v_k)) with per-partition bias
            si = 0
            for blk in range(n_dve, NBLK):
                S = schedule[blk]
                ci, c0 = chunk_of[blk]
                xg = ctiles[ci]
                nc.scalar.activation(
                    v1[:, si : si + S], xg[:, c0 + V : c0 + V + S],
                    ACTF.Copy, bias=1.0, scale=1.0,
                )
                for k in range(S):
                    so = spool.tile([P, V], BF16, tag="so")
                    nc.scalar.activation(
                        so[:], xg[:, c0 : c0 + V], ACTF.Relu,
                        bias=v1[:, si : si + 1], scale=1.0,
                        accum_out=allc[:, n_dve + si : n_dve + si + 1],
                    )
                    si += 1

            # ---- end-of-core combine ----
            h1 = apool.tile([P, 1], F32, tag="h1")
            nc.vector.tensor_reduce(h1[:], allc[:], AXL.X, ALU.add)
            l1 = apool.tile([P, 1], F32, tag="l1")
            nc.vector.tensor_reduce(l1[:], lcols[:], AXL.X, ALU.add)
            # w = (0.3/0.7)*h1 + l1 per partition; host applies 0.7/(V*ROWS)
            w1 = apool.tile([P, 1], F32, tag="w1")
            nc.vector.scalar_tensor_tensor(
                w1[:], h1[:], MLM_W / BCE_W, l1[:], ALU.mult, ALU.add
            )
            wps = pspool.tile([1, 1], F32, tag="wps")
            nc.tensor.matmul(wps[:], ones[:], w1[:], start=True, stop=True)
            wsb = apool.tile([1, 1], F32, tag="wsb")
            nc.scalar.copy(wsb[:], wps[:])
            nc.sync.dma_start(out_dram.ap()[:, :], wsb[:])

    nc.compile()
    return nc


_NC_CACHE = {}


def _get_nc(schedule, W):
    key = (schedule, W)
    if key not in _NC_CACHE:
        _NC_CACHE[key] = build_nc(schedule, W)
    return _NC_CACHE[key]


def _shard(x, t):
    """npos-sorted (descending) round-robin shard.
    Returns (schedule, W, [per-core [P, NBLK*(V+W)] bf16 arrays])."""
    pos = t > 0.5
    npos = pos.sum(axis=1)
    order = np.argsort(-npos, kind="stable")
    npos_s = npos[order]
    schedule = tuple(max(1, int(npos_s[b * N_CORES * P])) for b in range(NBLK))
    W = max(2, (schedule[0] + 1) // 2 * 2)     # even table width >= max S
    CB = V + W

    xs = x[order]
    ps = pos[order]
    u = np.where(ps, np.float32(NEG), xs)
    pcols = np.argsort(~ps, axis=1, kind="stable")[:, :W]
    vals = -np.take_along_axis(xs, pcols, axis=1)
    valid = np.arange(W)[None, :] < npos_s[:, None]
    v = np.where(valid, vals, np.float32(NEG))
    slab = np.concatenate([u, v], axis=1).astype(ml_dtypes.bfloat16)  # [ROWS, CB]

    shards = []
    for c in range(N_CORES):
        s = slab[c::N_CORES]                      # [RPC, CB] desc-sorted
        s = s.reshape(NBLK, P, CB).transpose(1, 0, 2).reshape(P, NBLK * CB)
        shards.append(np.ascontiguousarray(s))
    return schedule, W, shards


def kernel(logits: np.ndarray, targets: np.ndarray) -> np.ndarray:
    x = np.asarray(logits, dtype=np.float32).reshape(ROWS, V)
    t = np.asarray(targets, dtype=np.float32).reshape(ROWS, V)
    schedule, W, shards = _shard(x, t)
    nc = _get_nc(schedule, W)
    in_maps = [{"xp": shards[c]} for c in range(N_CORES)]
    res = run_bass_kernel_spmd(nc, in_maps, list(range(N_CORES)))
    total = sum(float(res.results[c]["out"][0, 0]) for c in range(N_CORES))
    return np.float32(total * BCE_W / (V * ROWS))
